# revision 13
# baseline (speedup 1.0000x reference)
"""Trainium2 Bass kernel for nn_DecisionStep (TabNet decision step).

Computation (per reference):
  al  = GBN(a @ w_att.T + b_att)                      # ghost batchnorm, chunks of 128
  m   = sparsemax(al * prior_scales)                  # ascending-sort variant
  np_ = prior_scales * (GAMMA - m)
  h   = glu(x*m, s1); h = s05*(h + glu(h, s2))
  h   = s05*(h + glu(h, d1)); h = s05*(h + glu(h, d2))
  out = (h[:, :64], relu(h[:, 64:]), np_, m)

Device strategy:
  * Pure data parallelism: batch rows sharded across 8 cores (8192 rows each).
  * Feature-major on device ([features, batch] tiles); the host transposes
    inputs/outputs (layout marshalling only).
  * Rows are processed in groups of 1024 = 8 GBN chunks. The host permutes
    batch order inside each group so chunk c occupies columns {v*8 + c}; a
    single grouped tensor_reduce (strided, unmergeable AP) then produces all
    8 chunks' sums in one instruction.
  * GBN: Sigma_x via grouped DVE reduce; Sigma_x^2 via ACT Square (same
    activation-table set as Sigmoid -> no table reloads) + grouped reduce.
    rsqrt(var+eps) via Quake-III bit-trick seed + 2 Newton iterations
    (DVE int ops + GPSIMD fp ops), avoiding the Sqrt table set entirely.
  * Biases before GBN cancel exactly (train-mode GBN subtracts the mean).
    GBN is scale-invariant up to eps, so the sqrt(0.5) residual scalings fold
    into per-layer batchnorm constants (gamma) and one final output scale.
  * Sparsemax: for this operator's ascending-sort variant the support is the
    full row whenever 1 + 255*max(z) - sum(z) > 0 (margin ~362 on this data,
    verified on host), so tau = (sum(z)+1)/255 and m = relu(z - tau).
  * Matmuls in float32r (full-rate, ~tf32 precision).
"""

import os
import numpy as np
from contextlib import ExitStack

import concourse.bass as bass
import concourse.tile as tile
from concourse import bacc, mybir
from concourse.bass_utils import run_bass_kernel_spmd

N, D, NA, ND = 65536, 256, 64, 64
H = NA + ND                  # 128
VBS = 128
GAMMA = 1.5
EPS = 1e-5
NCORES = 8
NLOC = N // NCORES           # 8192
S05 = float(np.sqrt(np.float32(0.5)))
ALPHA = S05 ** 3             # final output scale

F32 = mybir.dt.float32
F32R = mybir.dt.float32r
U32 = mybir.dt.uint32
AF = mybir.ActivationFunctionType
OP = mybir.AluOpType
AX = mybir.AxisListType

CPG = 8                      # chunks (of 128 rows) per group
GB = CPG * VBS               # 1024 rows per group
MAGIC = 0x5F3759DF           # Quake rsqrt seed constant

# host-folded batchnorm constants, per layer L in (att, s1, s2, d1, d2):
#   bnp col 4L+0 / 4L+1 : 128*gamma_L*bn_w   (lo / hi feature tile)
#   bnp col 4L+2 / 4L+3 : 128*gamma_L*bn_b   (lo / hi feature tile)
# (gamma applies to the u-half = lo tile only)
LAYER_GAMMA = [1.0, 1.0, 1.0, float(np.sqrt(np.float32(2.0))), 2.0]


def _cv(t, c):
    """Chunk view: strided [128, VBS] slice of an interleaved [128, GB] tile."""
    return t[:].rearrange("p (v c) -> p c v", c=CPG)[:, c]


def _gv(t):
    """Grouped view [128, CPG, VBS] of an interleaved [128, GB] tile."""
    return t[:].rearrange("p (v c) -> p c v", c=CPG)


def _emit_stats(ctx, tc, pools, mm_tiles, bnp, layer, tag):
    """GBN stats for a pair of [128, GB] PSUM tiles (lo/hi feature halves).

    Returns (scale, sp): [128, 2, CPG] sbuf tiles such that
      normalized = x * scale + sp  (gamma folding included via bnp).
    """
    nc = tc.nc
    stat, sb = pools["stat"], pools["sb"]
    s1 = stat.tile([128, 2, CPG], F32, tag="s1s", name=f"s1s{tag}")
    s2 = stat.tile([128, 2, CPG], F32, tag="s2s", name=f"s2s{tag}")
    sq = sb.tile([128, GB], F32, tag="sqt", name=f"sqt{tag}")
    for t in range(2):
        nc.vector.tensor_reduce(s1[:, t], _gv(mm_tiles[t]), axis=AX.X, op=OP.add)
        nc.scalar.activation(sq[:], mm_tiles[t][:], AF.Square)
        nc.vector.tensor_reduce(s2[:, t], _gv(sq), axis=AX.X, op=OP.add)
    # y = 128*S2 - S1^2 = 128^2 * var
    q = stat.tile([128, 2, CPG], F32, tag="q", name=f"q{tag}")
    y = stat.tile([128, 2, CPG], F32, tag="y", name=f"y{tag}")
    nc.vector.tensor_tensor(q[:], s1[:], s1[:], op=OP.mult)
    nc.gpsimd.tensor_scalar(s2[:], s2[:], 128.0, None, op0=OP.mult)
    nc.vector.tensor_tensor(y[:], s2[:], q[:], op=OP.subtract)
    # rsqrt of (y + 128^2 eps), scaled by 2^32: seed = 2^-31.5 / bitfloat(
    # bits(y)>>1) (max rel err ~6%), then 2 Newton steps in a 2^-64-scaled
    # domain (all on DVE-int/reciprocal + GPSIMD; no ACT table switch).
    sd = stat.tile([128, 2, CPG], F32, tag="sd", name=f"sd{tag}")
    xh = stat.tile([128, 2, CPG], F32, tag="xh", name=f"xh{tag}")
    t0 = stat.tile([128, 2, CPG], F32, tag="t0", name=f"t0{tag}")
    nc.vector.tensor_scalar(sd[:].bitcast(U32), y[:].bitcast(U32), 1, None,
                            op0=OP.logical_shift_right)
    nc.vector.reciprocal(sd[:], sd[:])
    nc.gpsimd.tensor_scalar(sd[:], sd[:], 2.0 ** -31.5, None, op0=OP.mult)
    nc.gpsimd.tensor_scalar(xh[:], y[:], 2.0 ** -65,
                            (128.0 ** 2 * EPS) * (2.0 ** -65),
                            op0=OP.mult, op1=OP.add)
    for _ in range(2):
        nc.gpsimd.tensor_tensor(t0[:], sd[:], sd[:], op=OP.mult)
        nc.gpsimd.tensor_tensor(t0[:], t0[:], xh[:], op=OP.mult)
        nc.gpsimd.tensor_scalar(t0[:], t0[:], -1.0, 1.5, op0=OP.mult, op1=OP.add)
        nc.gpsimd.tensor_tensor(sd[:], sd[:], t0[:], op=OP.mult)
    # sd = rsqrt(128^2 (var+eps)) * 2^32 = inv_sd * 2^32/128
    scale = stat.tile([128, 2, CPG], F32, tag="scale", name=f"scale{tag}")
    sp = stat.tile([128, 2, CPG], F32, tag="sp", name=f"sp{tag}")
    for t in range(2):
        wcol = bnp[:, 4 * layer + t : 4 * layer + t + 1]
        bcol = bnp[:, 4 * layer + 2 + t : 4 * layer + 3 + t]
        # scale = (128 gamma bn_w) * sd
        nc.gpsimd.tensor_scalar(scale[:, t], sd[:, t], wcol, None, op0=OP.mult)
        # sp = (128 gamma bn_b - S1*scale)/128
        nc.gpsimd.tensor_tensor(sp[:, t], s1[:, t], scale[:, t], op=OP.mult)
        nc.gpsimd.tensor_scalar(sp[:, t], sp[:, t], -1.0, bcol,
                                op0=OP.mult, op1=OP.add)
    nc.gpsimd.tensor_scalar(sp[:], sp[:], 1.0 / 128.0, None, op0=OP.mult)
    return scale, sp


def _emit_glu_block(ctx, tc, pools, inp_tiles, w_tiles, bnp, layer, tag):
    """One GLU block: matmul -> GBN -> u*sigmoid(g). Returns [128, GB] sbuf."""
    nc = tc.nc
    ps_mm, sb = pools["ps_mm"], pools["sb"]
    u_ps = ps_mm.tile([128, GB], F32, tag="mm", name=f"u{tag}")
    g_ps = ps_mm.tile([128, GB], F32, tag="mm", name=f"g{tag}")
    nk = len(inp_tiles)
    for h in range(2):
        hs = slice(h * 512, (h + 1) * 512)
        for k in range(nk):
            nc.tensor.matmul(u_ps[:, hs], w_tiles[k][:, 0:128],
                             inp_tiles[k][:, hs], start=(k == 0), stop=(k == nk - 1))
        for k in range(nk):
            nc.tensor.matmul(g_ps[:, hs], w_tiles[k][:, 128:256],
                             inp_tiles[k][:, hs], start=(k == 0), stop=(k == nk - 1))
    scale, sp = _emit_stats(ctx, tc, pools, [u_ps, g_ps], bnp, layer, tag)
    sig = sb.tile([128, GB], F32, tag="sig", name=f"sig{tag}")
    un = sb.tile([128, GB], F32, tag="un", name=f"un{tag}")
    for c in range(CPG):
        nc.scalar.activation(_cv(sig, c), _cv(g_ps, c), AF.Sigmoid,
                             bias=sp[:, 1, c : c + 1], scale=scale[:, 1, c : c + 1])
        nc.vector.tensor_scalar(_cv(un, c), _cv(u_ps, c),
                                scale[:, 0, c : c + 1], sp[:, 0, c : c + 1],
                                op0=OP.mult, op1=OP.add)
    glu = sb.tile([128, GB], F32R, tag="glu", name=f"glu{tag}")
    nc.gpsimd.tensor_tensor(glu[:], un[:], sig[:], op=OP.mult)
    return glu


def _emit_program(nloc: int, fast_prior: bool):
    nc = bacc.Bacc()
    ngroups = nloc // GB

    aT_d = nc.dram_tensor("aT", [NA, nloc], F32R, kind="ExternalInput")
    xT_d = nc.dram_tensor("xT", [D, nloc], F32, kind="ExternalInput")
    if not fast_prior:
        priorT_d = nc.dram_tensor("priorT", [D, nloc], F32, kind="ExternalInput")
    watT_d = nc.dram_tensor("watT", [NA, D], F32R, kind="ExternalInput")
    ws1T_d = nc.dram_tensor("ws1T", [D, 2 * H], F32R, kind="ExternalInput")
    ws2T_d = nc.dram_tensor("ws2T", [H, 2 * H], F32R, kind="ExternalInput")
    wd1T_d = nc.dram_tensor("wd1T", [H, 2 * H], F32R, kind="ExternalInput")
    wd2T_d = nc.dram_tensor("wd2T", [H, 2 * H], F32R, kind="ExternalInput")
    bnp_d = nc.dram_tensor("bnp", [128, 20], F32, kind="ExternalInput")
    onescol_d = nc.dram_tensor("onescol", [128, 1], F32R, kind="ExternalInput")
    onesrow_d = nc.dram_tensor("onesrow", [1, 128], F32R, kind="ExternalInput")

    mT_d = nc.dram_tensor("mT", [D, nloc], F32, kind="ExternalOutput")
    npT_d = nc.dram_tensor("npT", [D, nloc], F32, kind="ExternalOutput")
    aoutT_d = nc.dram_tensor("aoutT", [NA, nloc], F32, kind="ExternalOutput")
    dT_d = nc.dram_tensor("dT", [ND, nloc], F32, kind="ExternalOutput")

    with ExitStack() as ctx:
        tc = ctx.enter_context(tile.TileContext(nc))
        const = ctx.enter_context(tc.tile_pool(name="const", bufs=1))
        sb = ctx.enter_context(tc.tile_pool(name="sb", bufs=2))
        stat = ctx.enter_context(tc.tile_pool(name="stat", bufs=2))
        ps_mm = ctx.enter_context(tc.tile_pool(name="psmm", bufs=3, space="PSUM"))
        ps_t = ctx.enter_context(tc.tile_pool(name="pst", bufs=1, space="PSUM"))
        pools = {"sb": sb, "stat": stat, "ps_mm": ps_mm}

        # ---- constants ----
        watT = const.tile([NA, D], F32R)
        nc.sync.dma_start(watT[:], watT_d[:])
        ws1T = [const.tile([128, 2 * H], F32R, tag=f"ws1T{k}", name=f"ws1T{k}")
                for k in range(2)]
        for k in range(2):
            nc.sync.dma_start(ws1T[k][:], ws1T_d[k * 128 : (k + 1) * 128, :])
        ws2T = const.tile([H, 2 * H], F32R)
        nc.sync.dma_start(ws2T[:], ws2T_d[:])
        wd1T = const.tile([H, 2 * H], F32R)
        nc.sync.dma_start(wd1T[:], wd1T_d[:])
        wd2T = const.tile([H, 2 * H], F32R)
        nc.sync.dma_start(wd2T[:], wd2T_d[:])
        bnp = const.tile([128, 20], F32)
        nc.sync.dma_start(bnp[:], bnp_d[:])
        onescol = const.tile([128, 1], F32R)
        nc.sync.dma_start(onescol[:], onescol_d[:])
        onesrow = const.tile([1, 128], F32R)
        nc.sync.dma_start(onesrow[:], onesrow_d[:])

        # Warmup: touch every stationary operand once on PE so later matmuls
        # carry at most one semaphore wait each.
        warm = ps_mm.tile([128, GB], F32, tag="mm", name="warm")
        for w in (watT, ws1T[0], ws1T[1], ws2T, wd1T, wd2T):
            nc.tensor.matmul(warm[:, 0:128], w[:, 0:128], w[:, 0:128],
                             start=True, stop=True)
        nc.tensor.matmul(warm[0:1, 0:128], onescol[:], ws2T[:, 0:128],
                         start=True, stop=True)
        nc.tensor.matmul(warm[:, 0:128], onesrow[:], onesrow[:],
                         start=True, stop=True)

        for g in range(ngroups):
            gsl = slice(g * GB, (g + 1) * GB)
            # ---- loads ----
            a_sb = sb.tile([NA, GB], F32R, tag="a", name="a_sb")
            nc.sync.dma_start(a_sb[:], aT_d[:, gsl])
            x_sb = [sb.tile([128, GB], F32, tag=f"x{t}", name=f"x{t}")
                    for t in range(2)]
            for t in range(2):
                nc.sync.dma_start(x_sb[t][:], xT_d[t * 128 : (t + 1) * 128, gsl])
            if not fast_prior:
                pr_sb = [sb.tile([128, GB], F32, tag=f"pr{t}", name=f"pr{t}")
                         for t in range(2)]
                for t in range(2):
                    nc.sync.dma_start(
                        pr_sb[t][:], priorT_d[t * 128 : (t + 1) * 128, gsl]
                    )

            # ---- attentive transformer ----
            al_ps = [ps_mm.tile([128, GB], F32, tag="mm", name=f"al{t}")
                     for t in range(2)]
            for t in range(2):
                for h in range(2):
                    hs = slice(h * 512, (h + 1) * 512)
                    nc.tensor.matmul(al_ps[t][:, hs],
                                     watT[:, t * 128 : (t + 1) * 128],
                                     a_sb[:, hs], start=True, stop=True)
            scale, sp = _emit_stats(ctx, tc, pools, al_ps, bnp, 0, "att")
            z_sb = [sb.tile([128, GB], F32R, tag=f"z{t}", name=f"z{t}")
                    for t in range(2)]
            for t in range(2):
                for c in range(CPG):
                    nc.vector.tensor_scalar(
                        _cv(z_sb[t], c), _cv(al_ps[t], c),
                        scale[:, t, c : c + 1], sp[:, t, c : c + 1],
                        op0=OP.mult, op1=OP.add,
                    )
            if not fast_prior:
                for t in range(2):
                    nc.vector.tensor_tensor(
                        z_sb[t][:], z_sb[t][:], pr_sb[t][:], op=OP.mult
                    )

            # ---- sparsemax (full-support fast form) ----
            T_ps = ps_t.tile([1, GB], F32, tag="T", name="T_ps")
            for h in range(2):
                hs = slice(h * 512, (h + 1) * 512)
                for t in range(2):
                    nc.tensor.matmul(T_ps[:, hs], onescol[:], z_sb[t][:, hs],
                                     start=(t == 0), stop=(t == 1))
            tau = stat.tile([1, GB], F32R, tag="tau", name="tau")
            nc.scalar.activation(tau[:], T_ps[:], AF.Copy,
                                 bias=-1.0 / 255.0, scale=-1.0 / 255.0)
            taub_ps = ps_mm.tile([128, GB], F32, tag="mm", name="taub_ps")
            for h in range(2):
                hs = slice(h * 512, (h + 1) * 512)
                nc.tensor.matmul(taub_ps[:, hs], onesrow[:], tau[:, hs],
                                 start=True, stop=True)
            m_sb = [sb.tile([128, GB], F32, tag=f"m{t}", name=f"m{t}")
                    for t in range(2)]
            np_sb = [sb.tile([128, GB], F32, tag=f"np{t}", name=f"np{t}")
                     for t in range(2)]
            xm_sb = [sb.tile([128, GB], F32R, tag=f"xm{t}", name=f"xm{t}")
                     for t in range(2)]
            for t in range(2):
                nc.vector.tensor_tensor(m_sb[t][:], z_sb[t][:], taub_ps[:],
                                        op=OP.add)
                nc.gpsimd.tensor_scalar(m_sb[t][:], m_sb[t][:], 0.0, None,
                                        op0=OP.max)
                nc.sync.dma_start(mT_d[t * 128 : (t + 1) * 128, gsl], m_sb[t][:])
                # new_prior = prior * (GAMMA - m)
                nc.gpsimd.tensor_scalar(np_sb[t][:], m_sb[t][:], -1.0, GAMMA,
                                        op0=OP.mult, op1=OP.add)
                if not fast_prior:
                    nc.vector.tensor_tensor(
                        np_sb[t][:], np_sb[t][:], pr_sb[t][:], op=OP.mult
                    )
                nc.sync.dma_start(npT_d[t * 128 : (t + 1) * 128, gsl], np_sb[t][:])
                nc.vector.tensor_tensor(xm_sb[t][:], x_sb[t][:], m_sb[t][:],
                                        op=OP.mult)

            # ---- feature transformer ----
            s1t = _emit_glu_block(ctx, tc, pools, xm_sb, ws1T, bnp, 1, "s1")
            g2 = _emit_glu_block(ctx, tc, pools, [s1t], [ws2T], bnp, 2, "s2")
            s2t = sb.tile([128, GB], F32R, tag="S2", name="s2t")
            nc.vector.tensor_tensor(s2t[:], s1t[:], g2[:], op=OP.add)
            g3 = _emit_glu_block(ctx, tc, pools, [s2t], [wd1T], bnp, 3, "d1")
            s3t = sb.tile([128, GB], F32R, tag="S3", name="s3t")
            nc.vector.tensor_tensor(s3t[:], s2t[:], g3[:], op=OP.add)
            g4 = _emit_glu_block(ctx, tc, pools, [s3t], [wd2T], bnp, 4, "d2")
            s4t = sb.tile([128, GB], F32, tag="S4", name="s4t")
            nc.vector.tensor_tensor(s4t[:], s3t[:], g4[:], op=OP.add)

            # ---- outputs ----
            out_sb = sb.tile([128, GB], F32, tag="out", name="out_sb")
            nc.gpsimd.tensor_scalar(out_sb[0:NA, :], s4t[0:NA, :], ALPHA, None,
                                    op0=OP.mult)
            nc.gpsimd.tensor_scalar(out_sb[NA:128, :], s4t[NA:128, :], ALPHA, 0.0,
                                    op0=OP.mult, op1=OP.max)
            nc.sync.dma_start(aoutT_d[:, gsl], out_sb[0:NA, :])
            nc.sync.dma_start(dT_d[:, gsl], out_sb[NA:128, :])
    nc.compile()
    return nc


_PROGRAM_CACHE = {}
LAST_RESULTS = None


def _get_program(nloc: int, fast_prior: bool):
    key = (nloc, fast_prior)
    if key not in _PROGRAM_CACHE:
        _PROGRAM_CACHE[key] = _emit_program(nloc, fast_prior)
    return _PROGRAM_CACHE[key]


def _perm(nloc: int):
    """Device column g*GB + v*CPG + c holds original row g*GB + c*VBS + v."""
    ngroups = nloc // GB
    base = np.arange(ngroups, dtype=np.int64)[:, None, None] * GB
    v = np.arange(VBS, dtype=np.int64)[None, :, None]
    c = np.arange(CPG, dtype=np.int64)[None, None, :]
    return (base + c * VBS + v).reshape(-1)


def _pack_bnp(inputs):
    bnp = np.zeros((128, 20), np.float32)
    layers = [
        ("bn_att_w", "bn_att_b"),
        ("bn_s1_w", "bn_s1_b"),
        ("bn_s2_w", "bn_s2_b"),
        ("bn_d1_w", "bn_d1_b"),
        ("bn_d2_w", "bn_d2_b"),
    ]
    for L, (wname, bname) in enumerate(layers):
        gam = LAYER_GAMMA[L]
        w = np.asarray(inputs[wname], np.float32)
        b = np.asarray(inputs[bname], np.float32)
        bnp[:, 4 * L + 0] = 128.0 * gam * w[0:128] * (2.0 ** -32)
        bnp[:, 4 * L + 1] = 128.0 * w[128:256] * (2.0 ** -32)
        bnp[:, 4 * L + 2] = 128.0 * gam * b[0:128]
        bnp[:, 4 * L + 3] = 128.0 * b[128:256]
    return bnp


def _check_full_support(a, w_att, bn_att_w, bn_att_b, prior):
    """Verify the sparsemax full-support condition 1 + 255*max(z) > sum(z)."""
    al = a @ w_att.T
    xc = al.reshape(-1, VBS, D)
    mu = xc.mean(1, keepdims=True)
    var = ((xc - mu) ** 2).mean(1, keepdims=True)
    al_n = ((xc - mu) / np.sqrt(var + EPS)).reshape(-1, D)
    z = (al_n * bn_att_w + bn_att_b) * prior
    margin = 1.0 + 255.0 * z.max(1) - z.sum(1)
    return margin.min()


def kernel(**inputs):
    a = np.asarray(inputs["a"], np.float32)
    x = np.asarray(inputs["x"], np.float32)
    prior = np.asarray(inputs["prior_scales"], np.float32)
    w_att = np.asarray(inputs["w_att"], np.float32)

    fast_prior = bool(np.all(prior == 1.0))
    margin = _check_full_support(
        a, w_att,
        np.asarray(inputs["bn_att_w"], np.float32),
        np.asarray(inputs["bn_att_b"], np.float32),
        prior,
    )
    assert margin > 1.0, (
        f"sparsemax fast path invalid for this data (margin={margin}); "
        "general sorting path not implemented"
    )

    nc = _get_program(NLOC, fast_prior)

    perm = _perm(NLOC)
    aT = np.ascontiguousarray(a.T)
    xT = np.ascontiguousarray(x.T)
    if not fast_prior:
        priorT = np.ascontiguousarray(prior.T)
    weights = {
        "watT": np.ascontiguousarray(w_att.T),
        "ws1T": np.ascontiguousarray(np.asarray(inputs["w_s1"], np.float32).T),
        "ws2T": np.ascontiguousarray(np.asarray(inputs["w_s2"], np.float32).T),
        "wd1T": np.ascontiguousarray(np.asarray(inputs["w_d1"], np.float32).T),
        "wd2T": np.ascontiguousarray(np.asarray(inputs["w_d2"], np.float32).T),
        "bnp": _pack_bnp(inputs),
        "onescol": np.ones((128, 1), np.float32),
        "onesrow": np.ones((1, 128), np.float32),
    }
    in_maps = []
    for cidx in range(NCORES):
        csl = slice(cidx * NLOC, (cidx + 1) * NLOC)
        im = {
            "aT": np.ascontiguousarray(aT[:, csl][:, perm]),
            "xT": np.ascontiguousarray(xT[:, csl][:, perm]),
            **weights,
        }
        if not fast_prior:
            im["priorT"] = np.ascontiguousarray(priorT[:, csl][:, perm])
        in_maps.append(im)

    kw = {}
    if os.environ.get("KERNEL_TRACE"):
        kw["trace"] = True
        td = os.environ.get("KERNEL_TRACE_DIR")
        if td:
            os.makedirs(td, exist_ok=True)
            kw["tmpdir"] = td
    bkr = run_bass_kernel_spmd(nc, in_maps, list(range(NCORES)), **kw)
    global LAST_RESULTS
    LAST_RESULTS = bkr
    res = bkr.results

    def gather(key):
        nf = res[0][key].shape[0]
        out = np.empty((N, nf), np.float32)
        for cidx in range(NCORES):
            dev = np.asarray(res[cidx][key]).T  # [NLOC, nf], permuted rows
            blk = out[cidx * NLOC : (cidx + 1) * NLOC]
            blk[perm] = dev
        return out

    m = gather("mT")
    new_prior = gather("npT")
    a_out = gather("aoutT")
    d = gather("dT")
    return a_out, d, new_prior, m
